# revision 16
# baseline (speedup 1.0000x reference)
"""AttentionBlock Trainium2 kernel (Bass/Tile), SPMD over 8 NeuronCores.

Problem (hardcoded): x [32, 256, 32, 32] fp32
  GroupNorm(8 groups, eps=1e-5, affine) -> 1x1 qkv conv [768,256] ->
  per-image attention over N=1024 pixels (C=256) -> 1x1 proj [256,256] ->
  residual add.

Sharding: pure data-parallel over batch: 4 images per core, weights
replicated, no collectives.

Per-image on-chip schedule (channels-on-partitions layout, bf16 matmuls
with fp32 PSUM accumulation and fp32 statistics):
  - GroupNorm stats via bn_stats/bn_aggr (per-channel, fp32), pooled over
    each group's 32 channels with a tiny mask-matmul, finalized at group
    level in fp32, then broadcast back to channels via a DRAM-bounce DMA.
  - norm_w/norm_b are folded into the qkv weights on the host, the qk
    1/sqrt(C) scale is folded into Wq/Wk, v/proj biases fold into one
    final per-channel bias.
  - Attention is computed transposed: S^T[k,q] = k^T q directly from the
    conv layout (no transposes anywhere), softmax without max-subtraction
    (|S| <= ~6 by construction), Z via a ones-matmul, O = v_T^T P in PSUM
    accumulation over k-blocks, 1/Z applied to O in fp32 via a
    DRAM-bounce broadcast of the reciprocal row.

The per-image work is software-pipelined at emission order so the
per-engine streams overlap across images: stats of image b+1 run on
DVE while PE is busy with attention of image b, and the 1/Z DRAM
bounce of each q-chunk is hidden under the next chunk's matmuls.
"""

from contextlib import ExitStack

import ml_dtypes
import numpy as np

import concourse.bass as bass
import concourse.tile as tile
from concourse import bacc
from concourse import mybir

F32 = mybir.dt.float32
BF16 = mybir.dt.bfloat16
AF = mybir.ActivationFunctionType
OP = mybir.AluOpType

B, C, H, W = 32, 256, 32, 32
N = H * W            # 1024
G = 8                # groups
EPS = 1e-5
NCORES = 8
BL = B // NCORES     # images per core
CT = C // 128        # channel tiles
NB = N // 128        # pixel blocks (k dim of attention)
QCH = N // 512       # 512-wide q chunks
P = 128
RZ_SPLIT = True


def build_program(use_bq: bool, use_bk: bool, use_bf: bool) -> bass.Bass:
    nc = bacc.Bacc()

    xs = nc.dram_tensor("xs", [BL, C, N], F32, kind="ExternalInput")
    wq = nc.dram_tensor("wq", [C, C], BF16, kind="ExternalInput")  # [c_in, c_out]
    wk = nc.dram_tensor("wk", [C, C], BF16, kind="ExternalInput")
    wv = nc.dram_tensor("wv", [C, C], BF16, kind="ExternalInput")
    wp = nc.dram_tensor("wp", [C, C], BF16, kind="ExternalInput")
    bq = nc.dram_tensor("bq", [C], F32, kind="ExternalInput")
    bk = nc.dram_tensor("bk", [C], F32, kind="ExternalInput")
    bf = nc.dram_tensor("bf", [C], F32, kind="ExternalInput")
    out = nc.dram_tensor("out", [BL, C, N], F32, kind="ExternalOutput")

    # Constant matrix for the group-stat pooling matmul (mean over each
    # group's 32 channels; 1/32 is exact in bf16).
    gmask_np = np.zeros((P, 4), np.float32)
    gmask_np[np.arange(P), np.arange(P) // 32] = 1.0 / 32.0
    gmask_d = nc.inline_tensor(gmask_np.astype(ml_dtypes.bfloat16), "gmask")

    with tile.TileContext(nc) as tc, ExitStack() as ctx:
        consts = ctx.enter_context(tc.tile_pool(name="consts", bufs=1))
        xpool = ctx.enter_context(tc.tile_pool(name="xp", bufs=3))
        hpool = ctx.enter_context(tc.tile_pool(name="hp", bufs=2))
        qpool = ctx.enter_context(tc.tile_pool(name="qp", bufs=2))
        kpool = ctx.enter_context(tc.tile_pool(name="kp", bufs=2))
        vpool = ctx.enter_context(tc.tile_pool(name="vp", bufs=2))
        ppool = ctx.enter_context(tc.tile_pool(name="pp", bufs=3))
        opool = ctx.enter_context(tc.tile_pool(name="op", bufs=2))
        spool = ctx.enter_context(tc.tile_pool(name="sp", bufs=2))
        rzpool = ctx.enter_context(tc.tile_pool(name="rzp", bufs=2))
        outp = ctx.enter_context(tc.tile_pool(name="outp", bufs=4))
        dram = ctx.enter_context(tc.tile_pool(name="dram", bufs=2, space="DRAM"))
        psw = ctx.enter_context(tc.tile_pool(name="psw", bufs=2, space="PSUM"))
        psO = ctx.enter_context(tc.tile_pool(name="psO", bufs=2, space="PSUM"))
        psz = ctx.enter_context(tc.tile_pool(name="psz", bufs=2, space="PSUM"))

        # --- constants ---
        gmask_sb = consts.tile([P, 4], BF16, tag="gmask")
        nc.sync.dma_start(out=gmask_sb, in_=gmask_d[:, :])
        bq_sb = consts.tile([P, CT], F32, tag="bq")
        nc.sync.dma_start(out=bq_sb, in_=bq[:].rearrange("(t p) -> p t", p=P))
        bk_sb = consts.tile([P, CT], F32, tag="bk")
        nc.sync.dma_start(out=bk_sb, in_=bk[:].rearrange("(t p) -> p t", p=P))
        bf_sb = consts.tile([P, CT], F32, tag="bf")
        nc.sync.dma_start(out=bf_sb, in_=bf[:].rearrange("(t p) -> p t", p=P))
        onesc_sb = consts.tile([P, 1], F32, tag="onesc")
        nc.vector.memset(onesc_sb, 1.0)
        eps_sb = consts.tile([P, 1], F32, tag="eps")
        nc.vector.memset(eps_sb, EPS)
        wq_sb = consts.tile([P, CT, C], BF16, tag="wq")
        wk_sb = consts.tile([P, CT, C], BF16, tag="wk")
        wv_sb = consts.tile([P, CT, C], BF16, tag="wv")
        wp_sb = consts.tile([P, CT, C], BF16, tag="wp")
        for t_sb, t_d in ((wq_sb, wq), (wk_sb, wk), (wv_sb, wv), (wp_sb, wp)):
            nc.sync.dma_start(
                out=t_sb, in_=t_d[:, :].rearrange("(t p) o -> p t o", p=P)
            )

        for _ in range(24):
            warm_ps = psw.tile([P, 512], F32, tag="w")
            nc.tensor.matmul(
                warm_ps[:, 0:256], lhsT=wq_sb[:, 0, 0:P],
                rhs=wq_sb[:, 0, 0:256], start=True, stop=True,
            )

        # Per-image state carried between pipeline phases.
        st = [dict() for _ in range(BL)]

        def phase_a(b):
            """Load x, GroupNorm stats -> per-channel (mean, rstd), h."""
            x_t = xpool.tile([P, CT, N], F32, tag="x")
            st[b]["x"] = x_t
            for ct in range(CT):
                nc.sync.dma_start(
                    out=x_t[:, ct, :], in_=xs[b, ct * P : (ct + 1) * P, :]
                )
            chst = spool.tile([P, 2 * CT], F32, tag="chst")
            for ct in range(CT):
                bnst = spool.tile([P, 2, 6], F32, tag="bnst")
                for s in range(2):
                    nc.vector.bn_stats(
                        out=bnst[:, s, :], in_=x_t[:, ct, s * 512 : (s + 1) * 512]
                    )
                nc.vector.bn_aggr(out=chst[:, 2 * ct : 2 * ct + 2], in_=bnst)
                msq = spool.tile([P, 1], F32, tag="msq")
                nc.vector.tensor_mul(
                    out=msq,
                    in0=chst[:, 2 * ct : 2 * ct + 1],
                    in1=chst[:, 2 * ct : 2 * ct + 1],
                )
                nc.vector.tensor_add(
                    out=chst[:, 2 * ct + 1 : 2 * ct + 2],
                    in0=chst[:, 2 * ct + 1 : 2 * ct + 2],
                    in1=msq,
                )
            chst_bf = spool.tile([P, 2 * CT], BF16, tag="chstbf")
            nc.vector.tensor_copy(out=chst_bf, in_=chst)
            gst_ps = psw.tile([4, 2 * CT], F32, tag="w")
            nc.tensor.matmul(
                gst_ps, lhsT=gmask_sb, rhs=chst_bf, start=True, stop=True
            )
            gst_sb = spool.tile([4, 2 * CT], F32, tag="gst")
            nc.vector.tensor_copy(out=gst_sb, in_=gst_ps)
            gvar = spool.tile([4, CT], F32, tag="gvar")
            for ct in range(CT):
                gmsq = spool.tile([4, 1], F32, tag="gmsq")
                nc.vector.tensor_mul(
                    out=gmsq,
                    in0=gst_sb[:, 2 * ct : 2 * ct + 1],
                    in1=gst_sb[:, 2 * ct : 2 * ct + 1],
                )
                nc.vector.tensor_tensor(
                    out=gvar[:, ct : ct + 1],
                    in0=gst_sb[:, 2 * ct + 1 : 2 * ct + 2],
                    in1=gmsq,
                    op=OP.subtract,
                )
            gsd = spool.tile([4, CT], F32, tag="gsd")
            nc.scalar.activation(
                out=gsd, in_=gvar, func=AF.Sqrt, bias=eps_sb[0:4], scale=1.0
            )
            grstd = spool.tile([4, CT], F32, tag="grstd")
            nc.vector.reciprocal(out=grstd, in_=gsd)
            gfin = spool.tile([4, 2 * CT], F32, tag="gfin")
            for ct in range(CT):
                nc.vector.tensor_copy(
                    out=gfin[:, 2 * ct : 2 * ct + 1],
                    in_=gst_sb[:, 2 * ct : 2 * ct + 1],
                )
                nc.vector.tensor_copy(
                    out=gfin[:, 2 * ct + 1 : 2 * ct + 2],
                    in_=grstd[:, ct : ct + 1],
                )
            gfin_d = dram.tile([4, 2 * CT], F32, tag="gfd")
            nc.sync.dma_start(out=gfin_d, in_=gfin)
            pcs = spool.tile([P, 2 * CT], F32, tag="pcs")
            for g in range(4):
                nc.sync.dma_start(
                    out=pcs[32 * g : 32 * (g + 1), :],
                    in_=gfin_d[g : g + 1, :].to_broadcast((32, 2 * CT)),
                )
            h_t = hpool.tile([P, CT, N], BF16, tag="h")
            st[b]["h"] = h_t
            for ct in range(CT):
                nc.vector.tensor_scalar(
                    out=h_t[:, ct, :],
                    in0=x_t[:, ct, :],
                    scalar1=pcs[:, 2 * ct : 2 * ct + 1],
                    scalar2=pcs[:, 2 * ct + 1 : 2 * ct + 2],
                    op0=OP.subtract,
                    op1=OP.mult,
                )

        def phase_b(b):
            """qkv 1x1 convs."""
            h_t = st[b]["h"]
            q_sb = qpool.tile([P, CT, N], BF16, tag="q")
            k_sb = kpool.tile([P, CT, N], BF16, tag="k")
            st[b]["q"], st[b]["k"] = q_sb, k_sb
            for dst, w_sb, b_sb, use_b, on_act in (
                (q_sb, wq_sb, bq_sb, use_bq, True),
                (k_sb, wk_sb, bk_sb, use_bk, False),
            ):
                for ct in range(CT):
                    for nch in range(2):
                        mm_ps = psw.tile([P, 512], F32, tag="w")
                        for kc in range(CT):
                            nc.tensor.matmul(
                                mm_ps,
                                lhsT=w_sb[:, kc, ct * P : (ct + 1) * P],
                                rhs=h_t[:, kc, nch * 512 : (nch + 1) * 512],
                                start=(kc == 0),
                                stop=(kc == CT - 1),
                            )
                        dst_ap = dst[:, ct, nch * 512 : (nch + 1) * 512]
                        if use_b:
                            nc.vector.tensor_scalar_add(
                                out=dst_ap, in0=mm_ps, scalar1=b_sb[:, ct : ct + 1]
                            )
                        elif on_act:
                            nc.scalar.activation(
                                out=dst_ap, in_=mm_ps, func=AF.Copy, bias=0.0,
                                scale=1.0,
                            )
                        else:
                            nc.vector.tensor_copy(out=dst_ap, in_=mm_ps)
            v_sb = vpool.tile([P, NB, C], BF16, tag="v")
            st[b]["v"] = v_sb
            for nb in range(NB):
                vv_ps = psw.tile([P, C], F32, tag="w")
                for kc in range(CT):
                    nc.tensor.matmul(
                        vv_ps,
                        lhsT=h_t[:, kc, nb * P : (nb + 1) * P],
                        rhs=wv_sb[:, kc, :],
                        start=(kc == 0),
                        stop=(kc == CT - 1),
                    )
                nc.vector.tensor_copy(out=v_sb[:, nb, :], in_=vv_ps)

        def phase_c(b, qc):
            """Attention core for one 512-wide q chunk: S, exp, Z, O, 1/Z."""
            q_sb, k_sb, v_sb = st[b]["q"], st[b]["k"], st[b]["v"]
            O_ps = psO.tile([P, CT, 512], F32, tag="O")
            zacc = rzpool.tile([P, 512], F32, tag="zacc")
            st[b]["O%d" % qc] = O_ps
            st[b]["zacc%d" % qc] = zacc
            for nb in range(NB):
                s_ps = psw.tile([P, 512], F32, tag="w")
                for kc in range(CT):
                    nc.tensor.matmul(
                        s_ps,
                        lhsT=k_sb[:, kc, nb * P : (nb + 1) * P],
                        rhs=q_sb[:, kc, qc * 512 : (qc + 1) * 512],
                        start=(kc == 0),
                        stop=(kc == CT - 1),
                    )
                p_sb = ppool.tile([P, 512], BF16, tag="p")
                nc.scalar.activation(
                    out=p_sb, in_=s_ps, func=AF.Exp, bias=0.0, scale=1.0
                )
                # Z partial sums accumulate on the otherwise-idle GpSimd.
                if nb == 0:
                    nc.gpsimd.tensor_copy(out=zacc, in_=p_sb)
                else:
                    nc.gpsimd.tensor_tensor(
                        out=zacc, in0=zacc, in1=p_sb, op=OP.add
                    )
                for ct in range(CT):
                    nc.tensor.matmul(
                        O_ps[:, ct, :],
                        lhsT=v_sb[:, nb, ct * P : (ct + 1) * P],
                        rhs=p_sb,
                        start=(nb == 0),
                        stop=(nb == NB - 1),
                    )
        def phase_rz(b, qc):
            zacc = st[b].pop("zacc%d" % qc)
            z_ps = psz.tile([1, 512], F32, tag="z")
            nc.tensor.matmul(z_ps, lhsT=onesc_sb, rhs=zacc, start=True, stop=True)
            # 1/Z with the row transposed to [128, 4] so the reciprocal
            # runs across lanes (a [1, 512] reciprocal costs ~4us on DVE).
            z_sb = rzpool.tile([1, 512], F32, tag="zsb")
            nc.scalar.activation(
                out=z_sb, in_=z_ps, func=AF.Copy, bias=0.0, scale=1.0
            )
            z_d = dram.tile([1, 512], F32, tag="zd")
            nc.sync.dma_start(out=z_d, in_=z_sb)
            zT_sb = rzpool.tile([P, 4], F32, tag="zT")
            nc.sync.dma_start(
                out=zT_sb, in_=z_d[0, :].rearrange("(p j) -> p j", j=4)
            )
            rzT_sb = rzpool.tile([P, 4], F32, tag="rzT")
            nc.vector.reciprocal(out=rzT_sb, in_=zT_sb)
            rz_d = dram.tile([1, 512], F32, tag="rzd")
            nc.sync.dma_start(
                out=rz_d[0, :].rearrange("(p j) -> p j", j=4), in_=rzT_sb
            )
            rzb_sb = rzpool.tile([P, 512], F32, tag="rzb")
            st[b]["rzb%d" % qc] = rzb_sb
            nc.sync.dma_start(out=rzb_sb, in_=rz_d[:, :].to_broadcast((P, 512)))

        def phase_d(b, qc):
            """Apply 1/Z, proj conv, residual add, store."""
            O_ps = st[b].pop("O%d" % qc)
            rzb_sb = st[b].pop("rzb%d" % qc)
            x_t = st[b]["x"]
            on_sb = opool.tile([P, CT, 512], BF16, tag="on")
            for ct in range(CT):
                nc.vector.tensor_mul(
                    out=on_sb[:, ct, :], in0=O_ps[:, ct, :], in1=rzb_sb
                )
            for ct in range(CT):
                pr_ps = psw.tile([P, 512], F32, tag="w")
                for kc in range(CT):
                    nc.tensor.matmul(
                        pr_ps,
                        lhsT=wp_sb[:, kc, ct * P : (ct + 1) * P],
                        rhs=on_sb[:, kc, :],
                        start=(kc == 0),
                        stop=(kc == CT - 1),
                    )
                o_sb = outp.tile([P, 512], F32, tag="o")
                xres = x_t[:, ct, qc * 512 : (qc + 1) * 512]
                if use_bf:
                    nc.vector.scalar_tensor_tensor(
                        out=o_sb,
                        in0=pr_ps,
                        scalar=bf_sb[:, ct : ct + 1],
                        in1=xres,
                        op0=OP.add,
                        op1=OP.add,
                    )
                else:
                    nc.vector.tensor_add(out=o_sb, in0=pr_ps, in1=xres)
                nc.sync.dma_start(
                    out=out[b, ct * P : (ct + 1) * P, qc * 512 : (qc + 1) * 512],
                    in_=o_sb,
                )

        # Software pipeline: hide the stats chain of image b+1 under the
        # attention of image b, and each q-chunk's 1/Z DRAM bounce under
        # the next chunk's matmuls.
        phase_a(0)
        pending = None
        for b in range(BL):
            phase_b(b)
            if b + 1 < BL:
                phase_a(b + 1)
            for qc in range(QCH):
                phase_c(b, qc)
                if RZ_SPLIT:
                    if pending is not None:
                        phase_d(*pending)
                    phase_rz(b, qc)
                else:
                    phase_rz(b, qc)
                    if pending is not None:
                        phase_d(*pending)
                pending = (b, qc)
        phase_d(*pending)
    nc.compile()
    return nc


def prepare(inputs):
    """Fold parameters on the host; return (program, per-core input maps)."""
    x = np.ascontiguousarray(np.asarray(inputs["x"], dtype=np.float32))
    norm_w = np.asarray(inputs["norm_w"], dtype=np.float32)
    norm_b = np.asarray(inputs["norm_b"], dtype=np.float32)
    qkv_w = np.asarray(inputs["qkv_w"], dtype=np.float32)
    qkv_b = np.asarray(inputs["qkv_b"], dtype=np.float32)
    proj_w = np.asarray(inputs["proj_w"], dtype=np.float32)
    proj_b = np.asarray(inputs["proj_b"], dtype=np.float32)

    # Fold the GroupNorm affine into qkv: qkv(h*w+b) = (qkv*w)h + qkv@b
    w_eff = qkv_w * norm_w[None, :]
    b_eff = qkv_b + qkv_w @ norm_b
    s4 = float(C) ** -0.25  # sqrt of the attention 1/sqrt(C) scale
    bf16 = ml_dtypes.bfloat16
    wq_t = np.ascontiguousarray((w_eff[0:C] * s4).T.astype(bf16))
    wk_t = np.ascontiguousarray((w_eff[C : 2 * C] * s4).T.astype(bf16))
    wv_t = np.ascontiguousarray(w_eff[2 * C : 3 * C].T.astype(bf16))
    wp_t = np.ascontiguousarray(proj_w.T.astype(bf16))
    bq_f = np.ascontiguousarray(b_eff[0:C] * s4)
    bk_f = np.ascontiguousarray(b_eff[C : 2 * C] * s4)
    bv_f = b_eff[2 * C : 3 * C]
    bf_f = np.ascontiguousarray(proj_w @ bv_f + proj_b)

    use_bq = bool(np.any(bq_f))
    use_bk = bool(np.any(bk_f))
    use_bf = bool(np.any(bf_f))
    nc = build_program(use_bq, use_bk, use_bf)

    xr = x.reshape(NCORES, BL, C, N)
    in_maps = []
    for c in range(NCORES):
        in_maps.append(
            {
                "xs": np.ascontiguousarray(xr[c]),
                "wq": wq_t,
                "wk": wk_t,
                "wv": wv_t,
                "wp": wp_t,
                "bq": bq_f,
                "bk": bk_f,
                "bf": bf_f,
            }
        )
    return nc, in_maps


def run(inputs, trace=False):
    from concourse.bass_utils import run_bass_kernel_spmd

    nc, in_maps = prepare(inputs)
    res = run_bass_kernel_spmd(nc, in_maps, list(range(NCORES)), trace=trace)
    outs = np.stack([np.asarray(res.results[i]["out"]) for i in range(NCORES)])
    full = outs.reshape(B, C, H, W).astype(np.float32)
    return full, res


def kernel(**inputs) -> np.ndarray:
    full, _ = run(inputs, trace=False)
    return full


# revision 17
# speedup vs baseline: 1.0290x; 1.0290x over previous
"""AttentionBlock Trainium2 kernel (Bass/Tile), SPMD over 8 NeuronCores.

Problem (hardcoded): x [32, 256, 32, 32] fp32
  GroupNorm(8 groups, eps=1e-5, affine) -> 1x1 qkv conv [768,256] ->
  per-image attention over N=1024 pixels (C=256) -> 1x1 proj [256,256] ->
  residual add.

Sharding: pure data-parallel over batch: 4 images per core, weights
replicated, no collectives.

Per-image on-chip schedule (channels-on-partitions layout, bf16 matmuls
with fp32 PSUM accumulation and fp32 statistics):
  - GroupNorm stats via bn_stats/bn_aggr (per-channel, fp32), pooled over
    each group's 32 channels with a tiny mask-matmul, finalized at group
    level in fp32, then broadcast back to channels via a DRAM-bounce DMA.
  - norm_w/norm_b are folded into the qkv weights on the host, the qk
    1/sqrt(C) scale is folded into Wq/Wk, v/proj biases fold into one
    final per-channel bias.
  - Attention is computed transposed: S^T[k,q] = k^T q directly from the
    conv layout (no transposes anywhere), softmax without max-subtraction
    (|S| <= ~6 by construction), Z via a ones-matmul, O = v_T^T P in PSUM
    accumulation over k-blocks, 1/Z applied to O in fp32 via a
    DRAM-bounce broadcast of the reciprocal row.

The per-image work is software-pipelined at emission order so the
per-engine streams overlap across images: stats of image b+1 run on
DVE while PE is busy with attention of image b, and the 1/Z DRAM
bounce of each q-chunk is hidden under the next chunk's matmuls.
"""

from contextlib import ExitStack

import ml_dtypes
import numpy as np

import concourse.bass as bass
import concourse.tile as tile
from concourse import bacc
from concourse import mybir

F32 = mybir.dt.float32
BF16 = mybir.dt.bfloat16
AF = mybir.ActivationFunctionType
OP = mybir.AluOpType

B, C, H, W = 32, 256, 32, 32
N = H * W            # 1024
G = 8                # groups
EPS = 1e-5
NCORES = 8
BL = B // NCORES     # images per core
CT = C // 128        # channel tiles
NB = N // 128        # pixel blocks (k dim of attention)
QCH = N // 512       # 512-wide q chunks
P = 128
RZ_SPLIT = True


def build_program(use_bq: bool, use_bk: bool, use_bf: bool) -> bass.Bass:
    nc = bacc.Bacc()

    xs = nc.dram_tensor("xs", [BL, C, N], F32, kind="ExternalInput")
    wq = nc.dram_tensor("wq", [C, C], BF16, kind="ExternalInput")  # [c_in, c_out]
    wk = nc.dram_tensor("wk", [C, C], BF16, kind="ExternalInput")
    wv = nc.dram_tensor("wv", [C, C], BF16, kind="ExternalInput")
    wp = nc.dram_tensor("wp", [C, C], BF16, kind="ExternalInput")
    bq = nc.dram_tensor("bq", [C], F32, kind="ExternalInput")
    bk = nc.dram_tensor("bk", [C], F32, kind="ExternalInput")
    bf = nc.dram_tensor("bf", [C], F32, kind="ExternalInput")
    out = nc.dram_tensor("out", [BL, C, N], F32, kind="ExternalOutput")

    # Constant matrix for the group-stat pooling matmul (mean over each
    # group's 32 channels; 1/32 is exact in bf16).
    gmask_np = np.zeros((P, 4), np.float32)
    gmask_np[np.arange(P), np.arange(P) // 32] = 1.0 / 32.0
    gmask_d = nc.inline_tensor(gmask_np.astype(ml_dtypes.bfloat16), "gmask")

    with tile.TileContext(nc) as tc, ExitStack() as ctx:
        consts = ctx.enter_context(tc.tile_pool(name="consts", bufs=1))
        xpool = ctx.enter_context(tc.tile_pool(name="xp", bufs=3))
        hpool = ctx.enter_context(tc.tile_pool(name="hp", bufs=2))
        qpool = ctx.enter_context(tc.tile_pool(name="qp", bufs=2))
        kpool = ctx.enter_context(tc.tile_pool(name="kp", bufs=2))
        vpool = ctx.enter_context(tc.tile_pool(name="vp", bufs=2))
        ppool = ctx.enter_context(tc.tile_pool(name="pp", bufs=3))
        opool = ctx.enter_context(tc.tile_pool(name="op", bufs=2))
        spool = ctx.enter_context(tc.tile_pool(name="sp", bufs=2))
        rzpool = ctx.enter_context(tc.tile_pool(name="rzp", bufs=2))
        outp = ctx.enter_context(tc.tile_pool(name="outp", bufs=4))
        dram = ctx.enter_context(tc.tile_pool(name="dram", bufs=2, space="DRAM"))
        psw = ctx.enter_context(tc.tile_pool(name="psw", bufs=2, space="PSUM"))
        psO = ctx.enter_context(tc.tile_pool(name="psO", bufs=2, space="PSUM"))
        psz = ctx.enter_context(tc.tile_pool(name="psz", bufs=2, space="PSUM"))

        # --- constants ---
        gmask_sb = consts.tile([P, 4], BF16, tag="gmask")
        nc.sync.dma_start(out=gmask_sb, in_=gmask_d[:, :])
        bq_sb = consts.tile([P, CT], F32, tag="bq")
        nc.sync.dma_start(out=bq_sb, in_=bq[:].rearrange("(t p) -> p t", p=P))
        bk_sb = consts.tile([P, CT], F32, tag="bk")
        nc.sync.dma_start(out=bk_sb, in_=bk[:].rearrange("(t p) -> p t", p=P))
        bf_sb = consts.tile([P, CT], F32, tag="bf")
        nc.sync.dma_start(out=bf_sb, in_=bf[:].rearrange("(t p) -> p t", p=P))
        onesc_sb = consts.tile([P, 1], F32, tag="onesc")
        nc.vector.memset(onesc_sb, 1.0)
        eps_sb = consts.tile([P, 1], F32, tag="eps")
        nc.vector.memset(eps_sb, EPS)
        wq_sb = consts.tile([P, CT, C], BF16, tag="wq")
        wk_sb = consts.tile([P, CT, C], BF16, tag="wk")
        wv_sb = consts.tile([P, CT, C], BF16, tag="wv")
        wp_sb = consts.tile([P, CT, C], BF16, tag="wp")
        for t_sb, t_d in ((wq_sb, wq), (wk_sb, wk), (wv_sb, wv), (wp_sb, wp)):
            nc.sync.dma_start(
                out=t_sb, in_=t_d[:, :].rearrange("(t p) o -> p t o", p=P)
            )

        for _ in range(24):
            warm_ps = psw.tile([P, 512], F32, tag="w")
            nc.tensor.matmul(
                warm_ps[:, 0:256], lhsT=wq_sb[:, 0, 0:P],
                rhs=wq_sb[:, 0, 0:256], start=True, stop=True,
            )

        # Per-image state carried between pipeline phases.
        st = [dict() for _ in range(BL)]

        def phase_a(b):
            """Load x, GroupNorm stats -> per-channel (mean, rstd), h."""
            x_t = xpool.tile([P, CT, N], F32, tag="x")
            st[b]["x"] = x_t
            for ct in range(CT):
                nc.sync.dma_start(
                    out=x_t[:, ct, :], in_=xs[b, ct * P : (ct + 1) * P, :]
                )
            chst = spool.tile([P, 2 * CT], F32, tag="chst")
            for ct in range(CT):
                bnst = spool.tile([P, 2, 6], F32, tag="bnst")
                for s in range(2):
                    nc.vector.bn_stats(
                        out=bnst[:, s, :], in_=x_t[:, ct, s * 512 : (s + 1) * 512]
                    )
                nc.vector.bn_aggr(out=chst[:, 2 * ct : 2 * ct + 2], in_=bnst)
                msq = spool.tile([P, 1], F32, tag="msq")
                nc.vector.tensor_mul(
                    out=msq,
                    in0=chst[:, 2 * ct : 2 * ct + 1],
                    in1=chst[:, 2 * ct : 2 * ct + 1],
                )
                nc.vector.tensor_add(
                    out=chst[:, 2 * ct + 1 : 2 * ct + 2],
                    in0=chst[:, 2 * ct + 1 : 2 * ct + 2],
                    in1=msq,
                )
            chst_bf = spool.tile([P, 2 * CT], BF16, tag="chstbf")
            nc.vector.tensor_copy(out=chst_bf, in_=chst)
            gst_ps = psw.tile([4, 2 * CT], F32, tag="w")
            nc.tensor.matmul(
                gst_ps, lhsT=gmask_sb, rhs=chst_bf, start=True, stop=True
            )
            gst_sb = spool.tile([4, 2 * CT], F32, tag="gst")
            nc.vector.tensor_copy(out=gst_sb, in_=gst_ps)
            gvar = spool.tile([4, CT], F32, tag="gvar")
            for ct in range(CT):
                gmsq = spool.tile([4, 1], F32, tag="gmsq")
                nc.vector.tensor_mul(
                    out=gmsq,
                    in0=gst_sb[:, 2 * ct : 2 * ct + 1],
                    in1=gst_sb[:, 2 * ct : 2 * ct + 1],
                )
                nc.vector.tensor_tensor(
                    out=gvar[:, ct : ct + 1],
                    in0=gst_sb[:, 2 * ct + 1 : 2 * ct + 2],
                    in1=gmsq,
                    op=OP.subtract,
                )
            gsd = spool.tile([4, CT], F32, tag="gsd")
            nc.scalar.activation(
                out=gsd, in_=gvar, func=AF.Sqrt, bias=eps_sb[0:4], scale=1.0
            )
            grstd = spool.tile([4, CT], F32, tag="grstd")
            nc.vector.reciprocal(out=grstd, in_=gsd)
            gfin = spool.tile([4, 2 * CT], F32, tag="gfin")
            for ct in range(CT):
                nc.vector.tensor_copy(
                    out=gfin[:, 2 * ct : 2 * ct + 1],
                    in_=gst_sb[:, 2 * ct : 2 * ct + 1],
                )
                nc.vector.tensor_copy(
                    out=gfin[:, 2 * ct + 1 : 2 * ct + 2],
                    in_=grstd[:, ct : ct + 1],
                )
            gfin_d = dram.tile([4, 2 * CT], F32, tag="gfd")
            nc.sync.dma_start(out=gfin_d, in_=gfin)
            pcs = spool.tile([P, 2 * CT], F32, tag="pcs")
            for g in range(4):
                nc.sync.dma_start(
                    out=pcs[32 * g : 32 * (g + 1), :],
                    in_=gfin_d[g : g + 1, :].to_broadcast((32, 2 * CT)),
                )
            h_t = hpool.tile([P, CT, N], BF16, tag="h")
            st[b]["h"] = h_t
            for ct in range(CT):
                nc.vector.tensor_scalar(
                    out=h_t[:, ct, :],
                    in0=x_t[:, ct, :],
                    scalar1=pcs[:, 2 * ct : 2 * ct + 1],
                    scalar2=pcs[:, 2 * ct + 1 : 2 * ct + 2],
                    op0=OP.subtract,
                    op1=OP.mult,
                )

        def phase_b(b):
            """qkv 1x1 convs."""
            h_t = st[b]["h"]
            q_sb = qpool.tile([P, CT, N], BF16, tag="q")
            k_sb = kpool.tile([P, CT, N], BF16, tag="k")
            st[b]["q"], st[b]["k"] = q_sb, k_sb
            for dst, w_sb, b_sb, use_b, on_act in (
                (q_sb, wq_sb, bq_sb, use_bq, True),
                (k_sb, wk_sb, bk_sb, use_bk, False),
            ):
                for ct in range(CT):
                    for nch in range(2):
                        mm_ps = psw.tile([P, 512], F32, tag="w")
                        for kc in range(CT):
                            nc.tensor.matmul(
                                mm_ps,
                                lhsT=w_sb[:, kc, ct * P : (ct + 1) * P],
                                rhs=h_t[:, kc, nch * 512 : (nch + 1) * 512],
                                start=(kc == 0),
                                stop=(kc == CT - 1),
                            )
                        dst_ap = dst[:, ct, nch * 512 : (nch + 1) * 512]
                        if use_b:
                            nc.vector.tensor_scalar_add(
                                out=dst_ap, in0=mm_ps, scalar1=b_sb[:, ct : ct + 1]
                            )
                        elif on_act:
                            nc.scalar.activation(
                                out=dst_ap, in_=mm_ps, func=AF.Copy, bias=0.0,
                                scale=1.0,
                            )
                        else:
                            nc.vector.tensor_copy(out=dst_ap, in_=mm_ps)
            v_sb = vpool.tile([P, NB, C], BF16, tag="v")
            st[b]["v"] = v_sb
            for nb in range(NB):
                vv_ps = psw.tile([P, C], F32, tag="w")
                for kc in range(CT):
                    nc.tensor.matmul(
                        vv_ps,
                        lhsT=h_t[:, kc, nb * P : (nb + 1) * P],
                        rhs=wv_sb[:, kc, :],
                        start=(kc == 0),
                        stop=(kc == CT - 1),
                    )
                nc.vector.tensor_copy(out=v_sb[:, nb, :], in_=vv_ps)

        def phase_c(b, qc):
            """Attention core for one 512-wide q chunk: S, exp, Z, O, 1/Z."""
            q_sb, k_sb, v_sb = st[b]["q"], st[b]["k"], st[b]["v"]
            O_ps = psO.tile([P, CT, 512], F32, tag="O")
            zacc_g = rzpool.tile([P, 512], F32, tag="zaccg")
            zacc_v = rzpool.tile([P, 512], F32, tag="zaccv")
            st[b]["zacc%d" % qc] = (zacc_g, zacc_v)
            st[b]["O%d" % qc] = O_ps
            for nb in range(NB):
                s_ps = psw.tile([P, 512], F32, tag="w")
                for kc in range(CT):
                    nc.tensor.matmul(
                        s_ps,
                        lhsT=k_sb[:, kc, nb * P : (nb + 1) * P],
                        rhs=q_sb[:, kc, qc * 512 : (qc + 1) * 512],
                        start=(kc == 0),
                        stop=(kc == CT - 1),
                    )
                p_sb = ppool.tile([P, 512], BF16, tag="p")
                nc.scalar.activation(
                    out=p_sb, in_=s_ps, func=AF.Exp, bias=0.0, scale=1.0
                )
                # Z partial sums accumulate off the PE: the first half of
                # the k-blocks on GpSimd, the second half on DVE, so neither
                # chain straggles past the chunk's matmuls.
                if nb == 0:
                    nc.gpsimd.tensor_copy(out=zacc_g, in_=p_sb)
                elif nb < NB // 2:
                    nc.gpsimd.tensor_tensor(
                        out=zacc_g, in0=zacc_g, in1=p_sb, op=OP.add
                    )
                elif nb == NB // 2:
                    nc.vector.tensor_copy(out=zacc_v, in_=p_sb)
                else:
                    nc.vector.tensor_tensor(
                        out=zacc_v, in0=zacc_v, in1=p_sb, op=OP.add
                    )
                for ct in range(CT):
                    nc.tensor.matmul(
                        O_ps[:, ct, :],
                        lhsT=v_sb[:, nb, ct * P : (ct + 1) * P],
                        rhs=p_sb,
                        start=(nb == 0),
                        stop=(nb == NB - 1),
                    )
        def phase_rz(b, qc):
            zacc_g, zacc_v = st[b].pop("zacc%d" % qc)
            nc.vector.tensor_add(out=zacc_v, in0=zacc_v, in1=zacc_g)
            z_ps = psz.tile([1, 512], F32, tag="z")
            nc.tensor.matmul(
                z_ps, lhsT=onesc_sb, rhs=zacc_v, start=True, stop=True
            )
            # 1/Z with the row transposed to [128, 4] so the reciprocal
            # runs across lanes (a [1, 512] reciprocal costs ~4us on DVE).
            z_sb = rzpool.tile([1, 512], F32, tag="zsb")
            nc.scalar.activation(
                out=z_sb, in_=z_ps, func=AF.Copy, bias=0.0, scale=1.0
            )
            z_d = dram.tile([1, 512], F32, tag="zd")
            nc.sync.dma_start(out=z_d, in_=z_sb)
            zT_sb = rzpool.tile([P, 4], F32, tag="zT")
            nc.sync.dma_start(
                out=zT_sb, in_=z_d[0, :].rearrange("(p j) -> p j", j=4)
            )
            rzT_sb = rzpool.tile([P, 4], F32, tag="rzT")
            nc.vector.reciprocal(out=rzT_sb, in_=zT_sb)
            rz_d = dram.tile([1, 512], F32, tag="rzd")
            nc.sync.dma_start(
                out=rz_d[0, :].rearrange("(p j) -> p j", j=4), in_=rzT_sb
            )
            rzb_sb = rzpool.tile([P, 512], F32, tag="rzb")
            st[b]["rzb%d" % qc] = rzb_sb
            nc.sync.dma_start(out=rzb_sb, in_=rz_d[:, :].to_broadcast((P, 512)))

        def phase_d(b, qc):
            """Apply 1/Z, proj conv, residual add, store."""
            O_ps = st[b].pop("O%d" % qc)
            rzb_sb = st[b].pop("rzb%d" % qc)
            x_t = st[b]["x"]
            on_sb = opool.tile([P, CT, 512], BF16, tag="on")
            for ct in range(CT):
                nc.vector.tensor_mul(
                    out=on_sb[:, ct, :], in0=O_ps[:, ct, :], in1=rzb_sb
                )
            for ct in range(CT):
                pr_ps = psw.tile([P, 512], F32, tag="w")
                for kc in range(CT):
                    nc.tensor.matmul(
                        pr_ps,
                        lhsT=wp_sb[:, kc, ct * P : (ct + 1) * P],
                        rhs=on_sb[:, kc, :],
                        start=(kc == 0),
                        stop=(kc == CT - 1),
                    )
                o_sb = outp.tile([P, 512], F32, tag="o")
                xres = x_t[:, ct, qc * 512 : (qc + 1) * 512]
                if use_bf:
                    nc.vector.scalar_tensor_tensor(
                        out=o_sb,
                        in0=pr_ps,
                        scalar=bf_sb[:, ct : ct + 1],
                        in1=xres,
                        op0=OP.add,
                        op1=OP.add,
                    )
                else:
                    nc.vector.tensor_add(out=o_sb, in0=pr_ps, in1=xres)
                nc.sync.dma_start(
                    out=out[b, ct * P : (ct + 1) * P, qc * 512 : (qc + 1) * 512],
                    in_=o_sb,
                )

        # Software pipeline: hide the stats chain of image b+1 under the
        # attention of image b, and each q-chunk's 1/Z DRAM bounce under
        # the next chunk's matmuls.
        phase_a(0)
        pending = None
        for b in range(BL):
            phase_b(b)
            if b + 1 < BL:
                phase_a(b + 1)
            for qc in range(QCH):
                phase_c(b, qc)
                if RZ_SPLIT:
                    if pending is not None:
                        phase_d(*pending)
                    phase_rz(b, qc)
                else:
                    phase_rz(b, qc)
                    if pending is not None:
                        phase_d(*pending)
                pending = (b, qc)
        phase_d(*pending)
    nc.compile()
    return nc


def prepare(inputs):
    """Fold parameters on the host; return (program, per-core input maps)."""
    x = np.ascontiguousarray(np.asarray(inputs["x"], dtype=np.float32))
    norm_w = np.asarray(inputs["norm_w"], dtype=np.float32)
    norm_b = np.asarray(inputs["norm_b"], dtype=np.float32)
    qkv_w = np.asarray(inputs["qkv_w"], dtype=np.float32)
    qkv_b = np.asarray(inputs["qkv_b"], dtype=np.float32)
    proj_w = np.asarray(inputs["proj_w"], dtype=np.float32)
    proj_b = np.asarray(inputs["proj_b"], dtype=np.float32)

    # Fold the GroupNorm affine into qkv: qkv(h*w+b) = (qkv*w)h + qkv@b
    w_eff = qkv_w * norm_w[None, :]
    b_eff = qkv_b + qkv_w @ norm_b
    s4 = float(C) ** -0.25  # sqrt of the attention 1/sqrt(C) scale
    bf16 = ml_dtypes.bfloat16
    wq_t = np.ascontiguousarray((w_eff[0:C] * s4).T.astype(bf16))
    wk_t = np.ascontiguousarray((w_eff[C : 2 * C] * s4).T.astype(bf16))
    wv_t = np.ascontiguousarray(w_eff[2 * C : 3 * C].T.astype(bf16))
    wp_t = np.ascontiguousarray(proj_w.T.astype(bf16))
    bq_f = np.ascontiguousarray(b_eff[0:C] * s4)
    bk_f = np.ascontiguousarray(b_eff[C : 2 * C] * s4)
    bv_f = b_eff[2 * C : 3 * C]
    bf_f = np.ascontiguousarray(proj_w @ bv_f + proj_b)

    use_bq = bool(np.any(bq_f))
    use_bk = bool(np.any(bk_f))
    use_bf = bool(np.any(bf_f))
    nc = build_program(use_bq, use_bk, use_bf)

    xr = x.reshape(NCORES, BL, C, N)
    in_maps = []
    for c in range(NCORES):
        in_maps.append(
            {
                "xs": np.ascontiguousarray(xr[c]),
                "wq": wq_t,
                "wk": wk_t,
                "wv": wv_t,
                "wp": wp_t,
                "bq": bq_f,
                "bk": bk_f,
                "bf": bf_f,
            }
        )
    return nc, in_maps


def run(inputs, trace=False):
    from concourse.bass_utils import run_bass_kernel_spmd

    nc, in_maps = prepare(inputs)
    res = run_bass_kernel_spmd(nc, in_maps, list(range(NCORES)), trace=trace)
    outs = np.stack([np.asarray(res.results[i]["out"]) for i in range(NCORES)])
    full = outs.reshape(B, C, H, W).astype(np.float32)
    return full, res


def kernel(**inputs) -> np.ndarray:
    full, _ = run(inputs, trace=False)
    return full


# revision 20
# speedup vs baseline: 1.2256x; 1.1911x over previous
"""AttentionBlock Trainium2 kernel (Bass/Tile), SPMD over 8 NeuronCores.

Problem (hardcoded): x [32, 256, 32, 32] fp32
  GroupNorm(8 groups, eps=1e-5, affine) -> 1x1 qkv conv [768,256] ->
  per-image attention over N=1024 pixels (C=256) -> 1x1 proj [256,256] ->
  residual add.

Sharding: pure data-parallel over batch: 4 images per core, weights
replicated, no collectives.

Per-image on-chip schedule (channels-on-partitions layout, bf16 matmuls
with fp32 PSUM accumulation and fp32 statistics):
  - GroupNorm stats via bn_stats/bn_aggr (per-channel, fp32), pooled over
    each group's 32 channels with a tiny mask-matmul, finalized at group
    level in fp32, then broadcast back to channels via a DRAM-bounce DMA.
  - norm_w/norm_b are folded into the qkv weights on the host, the qk
    1/sqrt(C) scale is folded into Wq/Wk, v/proj biases fold into one
    final per-channel bias.
  - Attention is computed transposed: S^T[k,q] = k^T q directly from the
    conv layout (no transposes anywhere), softmax without max-subtraction
    (|S| <= ~6 by construction), Z via a ones-matmul, O = v_T^T P in PSUM
    accumulation over k-blocks, 1/Z applied to O in fp32 via a
    DRAM-bounce broadcast of the reciprocal row.

The per-image work is software-pipelined at emission order so the
per-engine streams overlap across images: stats of image b+1 run on
DVE while PE is busy with attention of image b, and the 1/Z DRAM
bounce of each q-chunk is hidden under the next chunk's matmuls.
"""

from contextlib import ExitStack

import ml_dtypes
import numpy as np

import concourse.bass as bass
import concourse.tile as tile
from concourse import bacc
from concourse import mybir

F32 = mybir.dt.float32
BF16 = mybir.dt.bfloat16
AF = mybir.ActivationFunctionType
OP = mybir.AluOpType

B, C, H, W = 32, 256, 32, 32
N = H * W            # 1024
G = 8                # groups
EPS = 1e-5
NCORES = 8
BL = B // NCORES     # images per core
CT = C // 128        # channel tiles
NB = N // 128        # pixel blocks (k dim of attention)
QCH = N // 512       # 512-wide q chunks
P = 128
RZ_SPLIT = True
import os as _os
Z_MODE = _os.environ.get("KERNEL_Z_MODE", "hybrid")  # 'pe' | 'hybrid'
N_WARM = int(_os.environ.get("KERNEL_N_WARM", "24"))


def build_program(use_bq: bool, use_bk: bool, use_bf: bool) -> bass.Bass:
    nc = bacc.Bacc()

    xs = nc.dram_tensor("xs", [BL, C, N], F32, kind="ExternalInput")
    wq = nc.dram_tensor("wq", [C, C], BF16, kind="ExternalInput")  # [c_in, c_out]
    wk = nc.dram_tensor("wk", [C, C], BF16, kind="ExternalInput")
    wv = nc.dram_tensor("wv", [C, C], BF16, kind="ExternalInput")
    wp = nc.dram_tensor("wp", [C, C], BF16, kind="ExternalInput")
    bq = nc.dram_tensor("bq", [C], F32, kind="ExternalInput")
    bk = nc.dram_tensor("bk", [C], F32, kind="ExternalInput")
    bf = nc.dram_tensor("bf", [C], F32, kind="ExternalInput")
    out = nc.dram_tensor("out", [BL, C, N], F32, kind="ExternalOutput")

    # Constant matrix for the group-stat pooling matmul (mean over each
    # group's 32 channels; 1/32 is exact in bf16).
    gmask_np = np.zeros((P, 4), np.float32)
    gmask_np[np.arange(P), np.arange(P) // 32] = 1.0 / 32.0
    gmask_d = nc.inline_tensor(gmask_np.astype(ml_dtypes.bfloat16), "gmask")

    with tile.TileContext(nc) as tc, ExitStack() as ctx:
        consts = ctx.enter_context(tc.tile_pool(name="consts", bufs=1))
        xpool = ctx.enter_context(tc.tile_pool(name="xp", bufs=3))
        hpool = ctx.enter_context(tc.tile_pool(name="hp", bufs=2))
        qpool = ctx.enter_context(tc.tile_pool(name="qp", bufs=2))
        kpool = ctx.enter_context(tc.tile_pool(name="kp", bufs=2))
        vpool = ctx.enter_context(tc.tile_pool(name="vp", bufs=2))
        ppool = ctx.enter_context(tc.tile_pool(name="pp", bufs=3))
        opool = ctx.enter_context(tc.tile_pool(name="op", bufs=2))
        spool = ctx.enter_context(tc.tile_pool(name="sp", bufs=2))
        rzpool = ctx.enter_context(tc.tile_pool(name="rzp", bufs=2))
        outp = ctx.enter_context(tc.tile_pool(name="outp", bufs=4))
        dram = ctx.enter_context(tc.tile_pool(name="dram", bufs=2, space="DRAM"))
        psw = ctx.enter_context(tc.tile_pool(name="psw", bufs=3, space="PSUM"))
        psO = ctx.enter_context(tc.tile_pool(name="psO", bufs=2, space="PSUM"))
        psz = ctx.enter_context(tc.tile_pool(name="psz", bufs=1, space="PSUM"))

        # --- constants ---
        gmask_sb = consts.tile([P, 4], BF16, tag="gmask")
        nc.sync.dma_start(out=gmask_sb, in_=gmask_d[:, :])
        bq_sb = consts.tile([P, CT], F32, tag="bq")
        nc.sync.dma_start(out=bq_sb, in_=bq[:].rearrange("(t p) -> p t", p=P))
        bk_sb = consts.tile([P, CT], F32, tag="bk")
        nc.sync.dma_start(out=bk_sb, in_=bk[:].rearrange("(t p) -> p t", p=P))
        bf_sb = consts.tile([P, CT], F32, tag="bf")
        nc.sync.dma_start(out=bf_sb, in_=bf[:].rearrange("(t p) -> p t", p=P))
        onesc_sb = consts.tile([P, 1], F32, tag="onesc")
        nc.vector.memset(onesc_sb, 1.0)
        onesc_bf_sb = consts.tile([P, 1], BF16, tag="onescbf")
        nc.vector.memset(onesc_bf_sb, 1.0)
        eps_sb = consts.tile([P, 1], F32, tag="eps")
        nc.vector.memset(eps_sb, EPS)
        wq_sb = consts.tile([P, CT, C], BF16, tag="wq")
        wk_sb = consts.tile([P, CT, C], BF16, tag="wk")
        wv_sb = consts.tile([P, CT, C], BF16, tag="wv")
        wp_sb = consts.tile([P, CT, C], BF16, tag="wp")
        for t_sb, t_d in ((wq_sb, wq), (wk_sb, wk), (wv_sb, wv), (wp_sb, wp)):
            nc.sync.dma_start(
                out=t_sb, in_=t_d[:, :].rearrange("(t p) o -> p t o", p=P)
            )

        for _ in range(N_WARM):
            warm_ps = psw.tile([P, 512], F32, tag="w")
            nc.tensor.matmul(
                warm_ps[:, 0:256], lhsT=wq_sb[:, 0, 0:P],
                rhs=wq_sb[:, 0, 0:256], start=True, stop=True,
            )

        # Per-image state carried between pipeline phases.
        st = [dict() for _ in range(BL)]

        def phase_a(b):
            """Load x, GroupNorm stats -> per-channel (mean, rstd), h."""
            x_t = xpool.tile([P, CT, N], F32, tag="x")
            st[b]["x"] = x_t
            for ct in range(CT):
                nc.sync.dma_start(
                    out=x_t[:, ct, :], in_=xs[b, ct * P : (ct + 1) * P, :]
                )
            chst = spool.tile([P, 2 * CT], F32, tag="chst")
            for ct in range(CT):
                bnst = spool.tile([P, 2, 6], F32, tag="bnst")
                for s in range(2):
                    nc.vector.bn_stats(
                        out=bnst[:, s, :], in_=x_t[:, ct, s * 512 : (s + 1) * 512]
                    )
                nc.vector.bn_aggr(out=chst[:, 2 * ct : 2 * ct + 2], in_=bnst)
                msq = spool.tile([P, 1], F32, tag="msq")
                nc.vector.tensor_mul(
                    out=msq,
                    in0=chst[:, 2 * ct : 2 * ct + 1],
                    in1=chst[:, 2 * ct : 2 * ct + 1],
                )
                nc.vector.tensor_add(
                    out=chst[:, 2 * ct + 1 : 2 * ct + 2],
                    in0=chst[:, 2 * ct + 1 : 2 * ct + 2],
                    in1=msq,
                )
            chst_bf = spool.tile([P, 2 * CT], BF16, tag="chstbf")
            nc.vector.tensor_copy(out=chst_bf, in_=chst)
            gst_ps = psw.tile([4, 2 * CT], F32, tag="w")
            nc.tensor.matmul(
                gst_ps, lhsT=gmask_sb, rhs=chst_bf, start=True, stop=True
            )
            gst_sb = spool.tile([4, 2 * CT], F32, tag="gst")
            nc.vector.tensor_copy(out=gst_sb, in_=gst_ps)
            gvar = spool.tile([4, CT], F32, tag="gvar")
            for ct in range(CT):
                gmsq = spool.tile([4, 1], F32, tag="gmsq")
                nc.vector.tensor_mul(
                    out=gmsq,
                    in0=gst_sb[:, 2 * ct : 2 * ct + 1],
                    in1=gst_sb[:, 2 * ct : 2 * ct + 1],
                )
                nc.vector.tensor_tensor(
                    out=gvar[:, ct : ct + 1],
                    in0=gst_sb[:, 2 * ct + 1 : 2 * ct + 2],
                    in1=gmsq,
                    op=OP.subtract,
                )
            gsd = spool.tile([4, CT], F32, tag="gsd")
            nc.scalar.activation(
                out=gsd, in_=gvar, func=AF.Sqrt, bias=eps_sb[0:4], scale=1.0
            )
            grstd = spool.tile([4, CT], F32, tag="grstd")
            nc.vector.reciprocal(out=grstd, in_=gsd)
            gfin = spool.tile([4, 2 * CT], F32, tag="gfin")
            for ct in range(CT):
                nc.vector.tensor_copy(
                    out=gfin[:, 2 * ct : 2 * ct + 1],
                    in_=gst_sb[:, 2 * ct : 2 * ct + 1],
                )
                nc.vector.tensor_copy(
                    out=gfin[:, 2 * ct + 1 : 2 * ct + 2],
                    in_=grstd[:, ct : ct + 1],
                )
            gfin_d = dram.tile([4, 2 * CT], F32, tag="gfd")
            nc.sync.dma_start(out=gfin_d, in_=gfin)
            pcs = spool.tile([P, 2 * CT], F32, tag="pcs")
            for g in range(4):
                nc.sync.dma_start(
                    out=pcs[32 * g : 32 * (g + 1), :],
                    in_=gfin_d[g : g + 1, :].to_broadcast((32, 2 * CT)),
                )
            h_t = hpool.tile([P, CT, N], BF16, tag="h")
            st[b]["h"] = h_t
            for ct in range(CT):
                nc.vector.tensor_scalar(
                    out=h_t[:, ct, :],
                    in0=x_t[:, ct, :],
                    scalar1=pcs[:, 2 * ct : 2 * ct + 1],
                    scalar2=pcs[:, 2 * ct + 1 : 2 * ct + 2],
                    op0=OP.subtract,
                    op1=OP.mult,
                )

        def phase_b(b):
            """qkv 1x1 convs."""
            h_t = st[b]["h"]
            q_sb = qpool.tile([P, CT, N], BF16, tag="q")
            k_sb = kpool.tile([P, CT, N], BF16, tag="k")
            st[b]["q"], st[b]["k"] = q_sb, k_sb
            for dst, w_sb, b_sb, use_b, on_act in (
                (q_sb, wq_sb, bq_sb, use_bq, True),
                (k_sb, wk_sb, bk_sb, use_bk, False),
            ):
                for ct in range(CT):
                    for nch in range(2):
                        mm_ps = psw.tile([P, 512], F32, tag="w")
                        for kc in range(CT):
                            nc.tensor.matmul(
                                mm_ps,
                                lhsT=w_sb[:, kc, ct * P : (ct + 1) * P],
                                rhs=h_t[:, kc, nch * 512 : (nch + 1) * 512],
                                start=(kc == 0),
                                stop=(kc == CT - 1),
                            )
                        dst_ap = dst[:, ct, nch * 512 : (nch + 1) * 512]
                        if use_b:
                            nc.vector.tensor_scalar_add(
                                out=dst_ap, in0=mm_ps, scalar1=b_sb[:, ct : ct + 1]
                            )
                        elif on_act:
                            nc.scalar.activation(
                                out=dst_ap, in_=mm_ps, func=AF.Copy, bias=0.0,
                                scale=1.0,
                            )
                        else:
                            nc.vector.tensor_copy(out=dst_ap, in_=mm_ps)
            v_sb = vpool.tile([P, NB, C], BF16, tag="v")
            st[b]["v"] = v_sb
            for nb in range(NB):
                vv_ps = psw.tile([P, C], F32, tag="w")
                for kc in range(CT):
                    nc.tensor.matmul(
                        vv_ps,
                        lhsT=h_t[:, kc, nb * P : (nb + 1) * P],
                        rhs=wv_sb[:, kc, :],
                        start=(kc == 0),
                        stop=(kc == CT - 1),
                    )
                nc.vector.tensor_copy(out=v_sb[:, nb, :], in_=vv_ps)

        def phase_c(b, qc):
            """Attention core for one 512-wide q chunk: S, exp, Z, O, 1/Z."""
            q_sb, k_sb, v_sb = st[b]["q"], st[b]["k"], st[b]["v"]
            O_ps = psO.tile([P, CT, 512], F32, tag="O")
            st[b]["O%d" % qc] = O_ps
            if Z_MODE == "pe":
                z_ps = psz.tile([1, 512], F32, tag="z")
                st[b]["zps%d" % qc] = z_ps
            else:
                zacc_g = rzpool.tile([P, 512], F32, tag="zaccg")
                zacc_v = rzpool.tile([P, 512], F32, tag="zaccv")
                st[b]["zacc%d" % qc] = (zacc_g, zacc_v)
            def s_matmul(nb):
                s_ps = psw.tile([P, 512], F32, tag="w", name="s_ps")
                for kc in range(CT):
                    nc.tensor.matmul(
                        s_ps,
                        lhsT=k_sb[:, kc, nb * P : (nb + 1) * P],
                        rhs=q_sb[:, kc, qc * 512 : (qc + 1) * 512],
                        start=(kc == 0),
                        stop=(kc == CT - 1),
                    )
                return s_ps

            # One-deep software pipeline: S(nb+1) is emitted before the
            # exp-gated z/O work of nb so the PE never waits on ScalarE.
            s_next = s_matmul(0)
            for nb in range(NB):
                s_ps = s_next
                if nb + 1 < NB:
                    s_next = s_matmul(nb + 1)
                p_sb = ppool.tile([P, 512], BF16, tag="p")
                nc.scalar.activation(
                    out=p_sb, in_=s_ps, func=AF.Exp, bias=0.0, scale=1.0
                )
                # Z partial sums accumulate off the PE: the first half of
                # the k-blocks on GpSimd, the second half on DVE, so neither
                # chain straggles past the chunk's matmuls.
                if Z_MODE == "pe":
                    nc.tensor.matmul(
                        st[b]["zps%d" % qc],
                        lhsT=onesc_bf_sb,
                        rhs=p_sb,
                        start=(nb == 0),
                        stop=(nb == NB - 1),
                    )
                elif nb == 0:
                    nc.gpsimd.tensor_copy(out=zacc_g, in_=p_sb)
                elif nb < NB // 2:
                    nc.gpsimd.tensor_tensor(
                        out=zacc_g, in0=zacc_g, in1=p_sb, op=OP.add
                    )
                elif nb == NB // 2:
                    nc.vector.tensor_copy(out=zacc_v, in_=p_sb)
                else:
                    nc.vector.tensor_tensor(
                        out=zacc_v, in0=zacc_v, in1=p_sb, op=OP.add
                    )
                for ct in range(CT):
                    nc.tensor.matmul(
                        O_ps[:, ct, :],
                        lhsT=v_sb[:, nb, ct * P : (ct + 1) * P],
                        rhs=p_sb,
                        start=(nb == 0),
                        stop=(nb == NB - 1),
                    )
        def phase_rz(b, qc):
            if Z_MODE == "pe":
                z_ps = st[b].pop("zps%d" % qc)
            else:
                zacc_g, zacc_v = st[b].pop("zacc%d" % qc)
                nc.vector.tensor_add(out=zacc_v, in0=zacc_v, in1=zacc_g)
                z_ps = psz.tile([1, 512], F32, tag="z")
                nc.tensor.matmul(
                    z_ps, lhsT=onesc_sb, rhs=zacc_v, start=True, stop=True
                )
            # 1/Z with the row transposed to [128, 4] so the reciprocal
            # runs across lanes (a [1, 512] reciprocal costs ~4us on DVE).
            z_sb = rzpool.tile([1, 512], F32, tag="zsb")
            nc.scalar.activation(
                out=z_sb, in_=z_ps, func=AF.Copy, bias=0.0, scale=1.0
            )
            z_d = dram.tile([1, 512], F32, tag="zd")
            nc.sync.dma_start(out=z_d, in_=z_sb)
            zT_sb = rzpool.tile([P, 4], F32, tag="zT")
            nc.sync.dma_start(
                out=zT_sb, in_=z_d[0, :].rearrange("(p j) -> p j", j=4)
            )
            rzT_sb = rzpool.tile([P, 4], F32, tag="rzT")
            nc.vector.reciprocal(out=rzT_sb, in_=zT_sb)
            rz_d = dram.tile([1, 512], F32, tag="rzd")
            nc.sync.dma_start(
                out=rz_d[0, :].rearrange("(p j) -> p j", j=4), in_=rzT_sb
            )
            rzb_sb = rzpool.tile([P, 512], F32, tag="rzb")
            st[b]["rzb%d" % qc] = rzb_sb
            nc.sync.dma_start(out=rzb_sb, in_=rz_d[:, :].to_broadcast((P, 512)))

        def phase_d(b, qc):
            """Apply 1/Z, proj conv, residual add, store."""
            O_ps = st[b].pop("O%d" % qc)
            rzb_sb = st[b].pop("rzb%d" % qc)
            x_t = st[b]["x"]
            on_sb = opool.tile([P, CT, 512], BF16, tag="on")
            for ct in range(CT):
                nc.vector.tensor_mul(
                    out=on_sb[:, ct, :], in0=O_ps[:, ct, :], in1=rzb_sb
                )
            for ct in range(CT):
                pr_ps = psw.tile([P, 512], F32, tag="w")
                for kc in range(CT):
                    nc.tensor.matmul(
                        pr_ps,
                        lhsT=wp_sb[:, kc, ct * P : (ct + 1) * P],
                        rhs=on_sb[:, kc, :],
                        start=(kc == 0),
                        stop=(kc == CT - 1),
                    )
                o_sb = outp.tile([P, 512], F32, tag="o")
                xres = x_t[:, ct, qc * 512 : (qc + 1) * 512]
                if use_bf:
                    nc.vector.scalar_tensor_tensor(
                        out=o_sb,
                        in0=pr_ps,
                        scalar=bf_sb[:, ct : ct + 1],
                        in1=xres,
                        op0=OP.add,
                        op1=OP.add,
                    )
                else:
                    nc.vector.tensor_add(out=o_sb, in0=pr_ps, in1=xres)
                nc.sync.dma_start(
                    out=out[b, ct * P : (ct + 1) * P, qc * 512 : (qc + 1) * 512],
                    in_=o_sb,
                )

        # Software pipeline: hide the stats chain of image b+1 under the
        # attention of image b, and each q-chunk's 1/Z DRAM bounce under
        # the next chunk's matmuls.
        phase_a(0)
        pending = None
        for b in range(BL):
            phase_b(b)
            if b + 1 < BL:
                phase_a(b + 1)
            for qc in range(QCH):
                phase_c(b, qc)
                if RZ_SPLIT:
                    if pending is not None:
                        phase_d(*pending)
                    phase_rz(b, qc)
                else:
                    phase_rz(b, qc)
                    if pending is not None:
                        phase_d(*pending)
                pending = (b, qc)
        phase_d(*pending)
    nc.compile()
    return nc


def prepare(inputs):
    """Fold parameters on the host; return (program, per-core input maps)."""
    x = np.ascontiguousarray(np.asarray(inputs["x"], dtype=np.float32))
    norm_w = np.asarray(inputs["norm_w"], dtype=np.float32)
    norm_b = np.asarray(inputs["norm_b"], dtype=np.float32)
    qkv_w = np.asarray(inputs["qkv_w"], dtype=np.float32)
    qkv_b = np.asarray(inputs["qkv_b"], dtype=np.float32)
    proj_w = np.asarray(inputs["proj_w"], dtype=np.float32)
    proj_b = np.asarray(inputs["proj_b"], dtype=np.float32)

    # Fold the GroupNorm affine into qkv: qkv(h*w+b) = (qkv*w)h + qkv@b
    w_eff = qkv_w * norm_w[None, :]
    b_eff = qkv_b + qkv_w @ norm_b
    s4 = float(C) ** -0.25  # sqrt of the attention 1/sqrt(C) scale
    bf16 = ml_dtypes.bfloat16
    wq_t = np.ascontiguousarray((w_eff[0:C] * s4).T.astype(bf16))
    wk_t = np.ascontiguousarray((w_eff[C : 2 * C] * s4).T.astype(bf16))
    wv_t = np.ascontiguousarray(w_eff[2 * C : 3 * C].T.astype(bf16))
    wp_t = np.ascontiguousarray(proj_w.T.astype(bf16))
    bq_f = np.ascontiguousarray(b_eff[0:C] * s4)
    bk_f = np.ascontiguousarray(b_eff[C : 2 * C] * s4)
    bv_f = b_eff[2 * C : 3 * C]
    bf_f = np.ascontiguousarray(proj_w @ bv_f + proj_b)

    use_bq = bool(np.any(bq_f))
    use_bk = bool(np.any(bk_f))
    use_bf = bool(np.any(bf_f))
    nc = build_program(use_bq, use_bk, use_bf)

    xr = x.reshape(NCORES, BL, C, N)
    in_maps = []
    for c in range(NCORES):
        in_maps.append(
            {
                "xs": np.ascontiguousarray(xr[c]),
                "wq": wq_t,
                "wk": wk_t,
                "wv": wv_t,
                "wp": wp_t,
                "bq": bq_f,
                "bk": bk_f,
                "bf": bf_f,
            }
        )
    return nc, in_maps


def run(inputs, trace=False):
    from concourse.bass_utils import run_bass_kernel_spmd

    nc, in_maps = prepare(inputs)
    res = run_bass_kernel_spmd(nc, in_maps, list(range(NCORES)), trace=trace)
    outs = np.stack([np.asarray(res.results[i]["out"]) for i in range(NCORES)])
    full = outs.reshape(B, C, H, W).astype(np.float32)
    return full, res


def kernel(**inputs) -> np.ndarray:
    full, _ = run(inputs, trace=False)
    return full
